# revision 1
# baseline (speedup 1.0000x reference)
"""ComplexBatchNorm2d (Trabelsi-style complex whitening BN) on 8 trn2 NeuronCores.

Sharding: over channels C (8 channels per core). Each channel's batch statistics
are computed entirely on one core, so no collectives are needed.

Per-core device kernel (Bass/Tile), fully channel-pipelined — each channel's
interleaved data is DMA'd into SBUF ONCE and used for both phases:
  stats:  accumulate the channel 2x2 Gram + plain sums via TensorE matmuls over
          interleaved [X|Y|1|0] chunk blocks (fp32r fast path); extract the
          three diagonals with identity-masked TT-mult + reduce; one ones-
          matmul folds partitions, giving the 5 raw sums on partition 0.
  2x2:    closed-form (V + eps I)^{-1/2} on partition 0, folded with
          gamma/beta into y_re = G00*xr + G01*xi + BR (same for im); the 6
          coefficients are broadcast to all partitions via a DRAM bounce.
  whiten: ScalarE computes both scaled terms per component, VectorE adds them
          writing (re, im) interleaved into SBUF; contiguous DMA out.
Channels overlap: while channel c whitens (ACT/DVE), channel c+1 runs its
Gram matmuls (PE) over prefetched data, keeping the DMA engines saturated.

Host side: slices/permutes inputs per core, builds the interleaved chunk
layout, gathers per-core outputs and permutes back to (B, C, H, W, 2).
"""

import numpy as np

# Problem geometry (hardcoded per contract).
B, C, H, W = 32, 64, 128, 128
NCORES = 8
CLOC = C // NCORES          # channels per core = 8
P = 128                     # SBUF partitions
N = B * H * W               # samples per channel = 524288
F = N // P                  # free columns per channel = 4096
CHUNK = 64                  # data columns per gram chunk
NCHUNK = F // CHUNK         # 64 chunks per channel
# [X(64) | Y(64) | ones(1) | zero-pad(1)] per chunk. The pad keeps every
# matmul moving-operand width even (fp32r FP32-HIGH-mode ISA restriction).
BLK = 2 * CHUNK + 2         # 130 cols per chunk
XYW = NCHUNK * BLK          # 8320 free cols per channel (interleaved layout)
YW = 2 * F                  # 8192 output cols per channel (re/im interleaved)
EPS = 1e-5

_CACHE = {}
_TRACE = False   # test.py sets this to capture NTFF profile / HW exec time
LAST = {}        # kernel() stores exec_time_ns etc. here

# tuning knobs (module-level so the bench harness can sweep them)
XY_BUFS = 4      # channel-data tiles in flight (each 33.3 KiB/partition... x4B)
WBLK = 16        # chunks per whitening block (16 -> quarter channel)


def _build_nc():
    import concourse.bacc as bacc
    import concourse.mybir as mybir
    from concourse.tile import TileContext, add_dep_helper

    f32 = mybir.dt.float32
    f32r = mybir.dt.float32r
    Alu = mybir.AluOpType
    Act = mybir.ActivationFunctionType
    Axis = mybir.AxisListType

    # Bacc (not raw Bass): Tile emits multi-wait sync_info that only the bacc
    # pipeline (nop/event-semaphore lowering) can legalize for walrus codegen.
    nc = bacc.Bacc("TRN2", target_bir_lowering=False)
    # xy carries float32 bits but is declared float32r end-to-end so the BIR
    # verifier accepts it as a (fast-path) FP32r matmul operand.
    xy_d = nc.declare_dram_parameter("xy", [CLOC, P, XYW], f32r, isOutput=False)
    consts_d = nc.declare_dram_parameter("consts", [P, CHUNK], f32, isOutput=False)
    gb_d = nc.declare_dram_parameter("gb", [P, 48], f32, isOutput=False)
    y_d = nc.declare_dram_parameter("y", [CLOC, P, YW], f32, isOutput=True)
    scratch_d = nc.dram_tensor("scratch", [CLOC, 6], f32)

    V = nc.vector
    HB = WBLK         # chunks per whitening block
    NW = NCHUNK // HB # whitening blocks per channel

    with TileContext(nc) as tc:
        with (
            tc.tile_pool(name="singles", bufs=1) as singles,
            tc.tile_pool(name="xyp", bufs=XY_BUFS) as xyp,
            tc.tile_pool(name="yp", bufs=2) as yp,
            tc.tile_pool(name="t1p", bufs=2) as t1p,
            tc.tile_pool(name="smallp", bufs=2) as smallp,
            tc.tile_pool(name="gramp", bufs=2, space="PSUM") as gramp,
            tc.tile_pool(name="spsum", bufs=2, space="PSUM") as spsump,
        ):
            consts = singles.tile([P, CHUNK], f32)
            nc.sync.dma_start(out=consts[:], in_=consts_d[:])
            gb = singles.tile([P, 48], f32)
            nc.sync.dma_start(out=gb[:], in_=gb_d[:])

            # DVE-staged identity (stacked 64x64 pair) so the masked-diag TT
            # ops depend on at most one cross-engine producer.
            ident = singles.tile([P, CHUNK], f32)
            V.tensor_copy(ident[:], consts[:])
            # Full 128-wide ones weights: fp32 matmuls must keep all PE column
            # groups active (col_grp==0xf), so M=1 lhsT is not encodable.
            ones_mat = singles.tile([P, P], f32)
            V.memset(ones_mat[:], 1.0)

            for c in range(CLOC):
                # ---- load this channel's interleaved data (used twice) ----
                xt = xyp.tile([P, XYW], f32r, tag="xy")
                ld = nc.sync.dma_start(out=xt[:], in_=xy_d[c])

                # ---- gram stats ----
                g = gramp.tile([P, 2 * BLK], f32, tag="gram")
                for j in range(NCHUNK):
                    w = 2 * BLK if j < NCHUNK - 1 else BLK
                    nc.tensor.matmul(
                        g[:, 0:w],
                        lhsT=xt[:, j * BLK: j * BLK + 2 * CHUNK],
                        rhs=xt[:, j * BLK: j * BLK + w],
                        start=(j == 0),
                        stop=(j == NCHUNK - 1),
                    )
                # g[0:64, 0:64]    = X^T X   (diag -> sum xr^2)
                # g[0:64, 64:128]  = X^T Y   (diag -> sum xr*xi)
                # g[64:128,64:128] = Y^T Y   (diag -> sum xi^2)
                # g[0:64, 128]     = col sums of X; g[64:128, 128] of Y
                stats = smallp.tile([P, 8], f32, tag="stats")
                V.memset(stats[:], 0.0)
                junk = smallp.tile([P, CHUNK], f32, tag="junk")
                V.tensor_mul(junk[0:CHUNK, :], g[0:CHUNK, 0:CHUNK],
                             ident[0:CHUNK, :])
                V.tensor_reduce(out=stats[0:CHUNK, 0:1], in_=junk[0:CHUNK, :],
                                axis=Axis.X, op=Alu.add)
                V.tensor_mul(junk[0:CHUNK, :], g[0:CHUNK, CHUNK:2 * CHUNK],
                             ident[0:CHUNK, :])
                V.tensor_reduce(out=stats[0:CHUNK, 1:2], in_=junk[0:CHUNK, :],
                                axis=Axis.X, op=Alu.add)
                V.tensor_mul(junk[CHUNK:P, :], g[CHUNK:P, CHUNK:2 * CHUNK],
                             ident[CHUNK:P, :])
                V.tensor_reduce(out=stats[CHUNK:P, 2:3], in_=junk[CHUNK:P, :],
                                axis=Axis.X, op=Alu.add)
                V.tensor_copy(stats[0:CHUNK, 3:4],
                              g[0:CHUNK, 2 * CHUNK: 2 * CHUNK + 1])
                V.tensor_copy(stats[CHUNK:P, 4:5],
                              g[CHUNK:P, 2 * CHUNK: 2 * CHUNK + 1])

                # partition fold: all 128 output rows hold the column sums
                s_ps = spsump.tile([P, 8], f32, tag="sps")
                nc.tensor.matmul(s_ps[:, :], lhsT=ones_mat[:], rhs=stats[:],
                                 start=True, stop=True)
                # Every psum row holds the same sums (ones weights), so
                # the 2x2 assembly runs on all partitions at once and the
                # resulting coefficients need no broadcast.
                s_sb = smallp.tile([P, 8], f32, tag="ssb")
                V.tensor_copy(s_sb[:], s_ps[:, :])

                # ---- 2x2 assembly, replicated across partitions ----
                SXX, SXY, SYY = s_sb[:, 0:1], s_sb[:, 1:2], s_sb[:, 2:3]
                SR, SI = s_sb[:, 3:4], s_sb[:, 4:5]
                tmp = smallp.tile([P, 16], f32, tag="tmp")

                def ts(i, tmp=tmp):
                    return tmp[:, i:i + 1]

                rN = 1.0 / N
                rN1 = 1.0 / (N - 1)
                MR, MI, u = ts(0), ts(1), ts(2)
                a, bb, cc = ts(3), ts(4), ts(5)
                V.tensor_scalar_mul(MR, SR, rN)
                V.tensor_scalar_mul(MI, SI, rN)
                # a=(Sxx-Sx*mr)/(N-1)+eps; b=(Sxy-Sx*mi)/(N-1);
                # c=(Syy-Sy*mi)/(N-1)+eps
                V.tensor_mul(u, SR, MR)
                V.tensor_sub(a, SXX, u)
                V.tensor_scalar(out=a, in0=a, scalar1=rN1, scalar2=EPS,
                                op0=Alu.mult, op1=Alu.add)
                V.tensor_mul(u, SR, MI)
                V.tensor_sub(bb, SXY, u)
                V.tensor_scalar_mul(bb, bb, rN1)
                V.tensor_mul(u, SI, MI)
                V.tensor_sub(cc, SYY, u)
                V.tensor_scalar(out=cc, in0=cc, scalar1=rN1, scalar2=EPS,
                                op0=Alu.mult, op1=Alu.add)
                # (M)^{-1/2} for M=[[a,b],[b,c]]: s=sqrt(ac-b^2);
                # t=sqrt(a+c+2s); W=[[c+s,-b],[-b,a+s]]/(s*t)
                det, s_, tr, st, inv = ts(6), ts(7), ts(8), ts(9), ts(10)
                V.tensor_mul(det, a, cc)
                V.tensor_mul(u, bb, bb)
                V.tensor_sub(det, det, u)
                nc.scalar.sqrt(s_, det)
                V.tensor_add(u, a, cc)
                V.tensor_scalar_mul(tr, s_, 2.0)
                V.tensor_add(tr, tr, u)
                nc.scalar.sqrt(tr, tr)
                V.tensor_mul(st, s_, tr)
                V.reciprocal(inv, st)
                w00, w01, w11, q = ts(11), ts(12), ts(13), ts(14)
                V.tensor_add(w00, cc, s_)
                V.tensor_mul(w00, w00, inv)
                V.scalar_tensor_tensor(out=w01, in0=bb, scalar=-1.0, in1=inv,
                                       op0=Alu.mult, op1=Alu.mult)
                V.tensor_add(w11, a, s_)
                V.tensor_mul(w11, w11, inv)
                # G = gamma @ W ; B' = beta - G @ mean
                g00 = gb[:, 0 * 8 + c: 0 * 8 + c + 1]
                g01 = gb[:, 1 * 8 + c: 1 * 8 + c + 1]
                g10 = gb[:, 2 * 8 + c: 2 * 8 + c + 1]
                g11 = gb[:, 3 * 8 + c: 3 * 8 + c + 1]
                br_ = gb[:, 4 * 8 + c: 4 * 8 + c + 1]
                bi_ = gb[:, 5 * 8 + c: 5 * 8 + c + 1]
                cb = smallp.tile([P, 6], f32, tag="cb")
                G00, G01, BR = cb[:, 0:1], cb[:, 1:2], cb[:, 2:3]
                G10, G11, BI = cb[:, 3:4], cb[:, 4:5], cb[:, 5:6]
                V.tensor_mul(q, g00, w00)
                V.scalar_tensor_tensor(out=G00, in0=w01, scalar=g01,
                                       in1=q, op0=Alu.mult, op1=Alu.add)
                V.tensor_mul(q, g00, w01)
                V.scalar_tensor_tensor(out=G01, in0=w11, scalar=g01,
                                       in1=q, op0=Alu.mult, op1=Alu.add)
                V.tensor_mul(q, g10, w00)
                V.scalar_tensor_tensor(out=G10, in0=w01, scalar=g11,
                                       in1=q, op0=Alu.mult, op1=Alu.add)
                V.tensor_mul(q, g10, w01)
                V.scalar_tensor_tensor(out=G11, in0=w11, scalar=g11,
                                       in1=q, op0=Alu.mult, op1=Alu.add)
                q2 = ts(15)
                V.tensor_mul(q, G00, MR)
                V.scalar_tensor_tensor(out=q2, in0=G01, scalar=MI,
                                       in1=q, op0=Alu.mult, op1=Alu.add)
                V.tensor_sub(BR, br_, q2)
                V.tensor_mul(q, G10, MR)
                V.scalar_tensor_tensor(out=q2, in0=G11, scalar=MI,
                                       in1=q, op0=Alu.mult, op1=Alu.add)
                V.tensor_sub(BI, bi_, q2)


                # Stage coefficients through ScalarE: the whiten ACT ops
                # read them as scale/bias operands, and same-engine program
                # order after this copy guarantees they are ready.

                # Bounce the coefficients through DRAM so the whiten ops
                # consume a DMA-produced tile (dependency-tracked path that
                # validated on hardware).
                nc.sync.dma_start(out=scratch_d[c:c + 1, :], in_=cb[0:1, :])
                cbB = smallp.tile([P, 6], f32, tag="cbB")
                nc.sync.dma_start(out=cbB[:],
                                  in_=scratch_d[c:c + 1, :].to_broadcast((P, 6)))

                # ---- whiten + affine, per half channel ----
                x3 = xt[:].bitcast(f32).rearrange("p (j k) -> p j k", k=BLK)
                for h in range(NW):
                    xr = x3[:, h * HB:(h + 1) * HB, 0:CHUNK]
                    xi = x3[:, h * HB:(h + 1) * HB, CHUNK:2 * CHUNK]
                    yt = yp.tile([P, HB, 2 * CHUNK], f32, tag="y")
                    t1 = t1p.tile([P, HB, CHUNK], f32, tag="t1")
                    t2 = t1p.tile([P, HB, CHUNK], f32, tag="t2")
                    i1 = V.tensor_scalar(out=t1[:], in0=xr,
                                         scalar1=cbB[:, 0:1], scalar2=cbB[:, 2:3],
                                         op0=Alu.mult, op1=Alu.add)
                    i2 = V.scalar_tensor_tensor(out=yt[:, :, 0:2 * CHUNK:2],
                                                in0=xi, scalar=cbB[:, 1:2],
                                                in1=t1[:], op0=Alu.mult,
                                                op1=Alu.add)
                    i3 = V.tensor_scalar(out=t2[:], in0=xr,
                                         scalar1=cbB[:, 3:4], scalar2=cbB[:, 5:6],
                                         op0=Alu.mult, op1=Alu.add)
                    i4 = V.scalar_tensor_tensor(out=yt[:, :, 1:2 * CHUNK:2],
                                                in0=xi, scalar=cbB[:, 4:5],
                                                in1=t2[:], op0=Alu.mult,
                                                op1=Alu.add)
                    nc.sync.dma_start(
                        out=y_d[c][:, h * HB * 2 * CHUNK:(h + 1) * HB * 2 * CHUNK],
                        in_=yt[:].rearrange("p a b -> p (a b)"))

    nc.finalize()
    return nc


def _get_nc():
    if "nc" not in _CACHE:
        _CACHE["nc"] = _build_nc()
    return _CACHE["nc"]


def _prep_consts():
    ident = np.zeros((P, CHUNK), np.float32)
    ident[np.arange(P), np.arange(P) % CHUNK] = 1.0
    return ident


def _prep_core(x_real, x_imag, gamma, beta, k):
    c0 = k * CLOC
    xr = np.ascontiguousarray(
        x_real[:, c0:c0 + CLOC].transpose(1, 0, 2, 3)
    ).reshape(CLOC, P, NCHUNK, CHUNK)
    xi = np.ascontiguousarray(
        x_imag[:, c0:c0 + CLOC].transpose(1, 0, 2, 3)
    ).reshape(CLOC, P, NCHUNK, CHUNK)
    xy = np.empty((CLOC, P, NCHUNK, BLK), np.float32)
    xy[..., 0:CHUNK] = xr
    xy[..., CHUNK:2 * CHUNK] = xi
    xy[..., 2 * CHUNK] = 1.0
    xy[..., 2 * CHUNK + 1] = 0.0
    g = gamma[c0:c0 + CLOC]
    b = beta[c0:c0 + CLOC]
    gb = np.concatenate([g[:, 0, 0], g[:, 0, 1], g[:, 1, 0], g[:, 1, 1],
                         b[:, 0], b[:, 1]]).astype(np.float32).reshape(1, 48)
    gb = np.broadcast_to(gb, (P, 48)).copy()
    return {"xy": xy.reshape(CLOC, P, XYW), "consts": _prep_consts(), "gb": gb}


def kernel(x_real, x_imag, gamma, beta):
    from concourse.bass_utils import run_bass_kernel_spmd

    x_real = np.asarray(x_real, dtype=np.float32)
    x_imag = np.asarray(x_imag, dtype=np.float32)
    gamma = np.asarray(gamma, dtype=np.float32)
    beta = np.asarray(beta, dtype=np.float32)

    in_maps = [_prep_core(x_real, x_imag, gamma, beta, k)
               for k in range(NCORES)]

    nc = _get_nc()
    res = None
    if _TRACE:
        try:
            res = run_bass_kernel_spmd(nc, in_maps, list(range(NCORES)),
                                       trace=True)
        except Exception as e:  # trace infra unavailable -> plain run
            LAST["trace_error"] = repr(e)
            res = None
    if res is None:
        res = run_bass_kernel_spmd(nc, in_maps, list(range(NCORES)))
    LAST["exec_time_ns"] = res.exec_time_ns
    LAST["mean_exec_time_ns"] = res.mean_exec_time_ns
    LAST["profile_json"] = res.profile_json

    out = np.empty((B, C, H, W, 2), np.float32)
    for k in range(NCORES):
        c0 = k * CLOC
        y = res.results[k]["y"].reshape(CLOC, N, 2).reshape(CLOC, B, H, W, 2)
        out[:, c0:c0 + CLOC] = y.transpose(1, 0, 2, 3, 4)
    return out



# revision 2
# speedup vs baseline: 1.4957x; 1.4957x over previous
"""ComplexBatchNorm2d (Trabelsi-style complex whitening BN) on 8 trn2 NeuronCores.

Sharding: over channels C (8 channels per core); each channel's batch stats are
computed wholly on one core, so no collectives.

bf16 end-to-end: inputs are rounded to bf16 on the host (round-to-nearest-even)
and shipped in a per-pair blocked-interleaved layout [Xr(64)|Xi(64)|1|0] x 64
chunks; outputs leave the device as bf16 planes and are widened to f32 on the
host. This halves HBM traffic (the kernel is memory-bound) and doubles DVE
throughput (2x packed modes) vs the f32 version. Well within the 2e-2 rel-err
gate (bf16 rounding contributes ~3e-3).

Per-core pipeline, processed in channel PAIRS so the small per-channel 2x2
solves batch two channels per DVE op:
  stats:  per channel, 64 bf16 TensorE matmuls accumulate the [X|Y]^T[X|Y|1]
          Gram + column sums into PSUM; one masked multiply + 3 reduces
          extract the five raw sums for both channels at once; a ones-matmul
          folds partitions so every partition holds the totals.
  2x2:    closed-form (V + eps I)^{-1/2} on all partitions, folded with
          gamma/beta into y_re = G00*xr + G01*xi + BR (same for im).
  whiten: ScalarE computes t = G*xr + B via Identity-activation (per-partition
          scale/bias APs), VectorE adds the xi term with a 2x-mode
          scalar_tensor_tensor, writing dense bf16 re/im planes; one 2.1 MB
          output DMA per channel (issued from GPSIMD/SWDGE so input HWDGE
          DMAs never queue behind output waits).

Host side: slices/permutes inputs per core, rounds to bf16, builds the blocked
layout, widens per-core outputs and permutes back to (B, C, H, W, 2) f32.
"""

import numpy as np
import ml_dtypes

# Problem geometry (hardcoded per contract).
B, C, H, W = 32, 64, 128, 128
NCORES = 8
CLOC = C // NCORES          # channels per core = 8
NPAIR = CLOC // 2           # channel pairs per core = 4
P = 128                     # SBUF partitions
N = B * H * W               # samples per channel = 524288
F = N // P                  # free columns per channel plane = 4096
CHUNK = 64                  # data columns per gram chunk
NCHUNK = F // CHUNK         # 64 chunks per channel
BLK = 2 * CHUNK + 2         # [Xr(64)|Xi(64)|1|pad] = 130 cols per chunk
XYW = NCHUNK * BLK          # 8320 cols per channel (blocked layout)
F2 = 2 * F                  # 8192 output cols per channel (re plane | im plane)
EPS = 1e-5

_CACHE = {}
_TRACE = False   # test.py sets this to capture NTFF profile / HW exec time
LAST = {}        # kernel() stores exec_time_ns etc. here


def _build_nc():
    import concourse.bacc as bacc
    import concourse.mybir as mybir
    from concourse.tile import TileContext

    f32 = mybir.dt.float32
    bf16 = mybir.dt.bfloat16
    Alu = mybir.AluOpType
    Act = mybir.ActivationFunctionType
    Axis = mybir.AxisListType

    nc = bacc.Bacc("TRN2", target_bir_lowering=False)
    xy_d = nc.declare_dram_parameter("xy", [NPAIR, P, 2 * XYW], bf16,
                                     isOutput=False)
    mask_d = nc.declare_dram_parameter("mask", [P, 256], f32, isOutput=False)
    gb_d = nc.declare_dram_parameter("gb", [P, 48], f32, isOutput=False)
    y_d = nc.declare_dram_parameter("y", [CLOC, P, F2], bf16, isOutput=True)

    V = nc.vector
    rN = 1.0 / N
    rN1 = 1.0 / (N - 1)

    with TileContext(nc) as tc:
        with (
            tc.tile_pool(name="singles", bufs=1) as singles,
            tc.tile_pool(name="xyp", bufs=3) as xyp,
            tc.tile_pool(name="yp", bufs=2) as yp,
            tc.tile_pool(name="tp", bufs=4) as tp,
            tc.tile_pool(name="smallp", bufs=2) as smallp,
            tc.tile_pool(name="gramp", bufs=2, space="PSUM") as gramp,
            tc.tile_pool(name="spsum", bufs=2, space="PSUM") as spsump,
        ):
            mask = singles.tile([P, 2, 128], f32)
            nc.sync.dma_start(out=mask[:].rearrange("p a b -> p (a b)"),
                              in_=mask_d[:])
            gb = singles.tile([P, 48], f32)
            nc.sync.dma_start(out=gb[:], in_=gb_d[:])
            # Full 128-wide ones weights for the partition-fold matmul
            # (fp32 matmuls must keep all PE column groups active).
            ones_mat = singles.tile([P, P], f32)
            V.memset(ones_mat[:], 1.0)

            for pr in range(NPAIR):
                # ---- load this pair's blocked data (used twice) ----
                xt = xyp.tile([P, 2, XYW], bf16, tag="xy")
                nc.sync.dma_start(out=xt[:].rearrange("p a b -> p (a b)"),
                                  in_=xy_d[pr])

                # ---- gram stats (both channels into one 2-bank psum) ----
                g2 = gramp.tile([P, 2, 512], f32, tag="gram")
                for i in range(2):
                    for j in range(NCHUNK):
                        nc.tensor.matmul(
                            g2[:, i, 0:BLK],
                            lhsT=xt[:, i, j * BLK: j * BLK + 2 * CHUNK],
                            rhs=xt[:, i, j * BLK: j * BLK + BLK],
                            start=(j == 0),
                            stop=(j == NCHUNK - 1),
                        )

                # ---- batched diag/sum extraction for the pair ----
                stats = smallp.tile([P, 8, 2], f32, tag="stats")
                V.memset(stats[:], 0.0)
                junk = smallp.tile([P, 2, 128], f32, tag="junk")
                V.tensor_mul(junk[:], g2[:, :, 0:128], mask[:])
                V.tensor_reduce(out=stats[:, 0, :], in_=junk[:, :, 0:CHUNK],
                                axis=Axis.X, op=Alu.add)
                V.tensor_reduce(out=stats[0:CHUNK, 1, :],
                                in_=junk[0:CHUNK, :, CHUNK:2 * CHUNK],
                                axis=Axis.X, op=Alu.add)
                V.tensor_reduce(out=stats[CHUNK:P, 2, :],
                                in_=junk[CHUNK:P, :, CHUNK:2 * CHUNK],
                                axis=Axis.X, op=Alu.add)
                V.tensor_copy(stats[0:CHUNK, 3, :],
                              g2[0:CHUNK, :, 2 * CHUNK])
                V.tensor_copy(stats[CHUNK:P, 4, :],
                              g2[CHUNK:P, :, 2 * CHUNK])

                # partition fold: every psum row ends up with the totals
                s_ps = spsump.tile([P, 16], f32, tag="sps")
                nc.tensor.matmul(s_ps[:], lhsT=ones_mat[:],
                                 rhs=stats[:].rearrange("p a b -> p (a b)"),
                                 start=True, stop=True)
                s_sb = smallp.tile([P, 8, 2], f32, tag="ssb")
                V.tensor_copy(s_sb[:].rearrange("p a b -> p (a b)"), s_ps[:])

                # ---- 2x2 assembly, replicated across partitions,
                #      batched over the channel pair (FD=2 vectors) ----
                SXX, SXY, SYY = s_sb[:, 0, :], s_sb[:, 1, :], s_sb[:, 2, :]
                SR, SI = s_sb[:, 3, :], s_sb[:, 4, :]
                tmp = smallp.tile([P, 16, 2], f32, tag="tmp")

                def ts(i, tmp=tmp):
                    return tmp[:, i, :]

                MR, MI, u = ts(0), ts(1), ts(2)
                a, bb, cc = ts(3), ts(4), ts(5)
                V.tensor_scalar_mul(MR, SR, rN)
                V.tensor_scalar_mul(MI, SI, rN)
                V.tensor_mul(u, SR, MR)
                V.tensor_sub(a, SXX, u)
                V.tensor_scalar(out=a, in0=a, scalar1=rN1, scalar2=EPS,
                                op0=Alu.mult, op1=Alu.add)
                V.tensor_mul(u, SR, MI)
                V.tensor_sub(bb, SXY, u)
                V.tensor_scalar_mul(bb, bb, rN1)
                V.tensor_mul(u, SI, MI)
                V.tensor_sub(cc, SYY, u)
                V.tensor_scalar(out=cc, in0=cc, scalar1=rN1, scalar2=EPS,
                                op0=Alu.mult, op1=Alu.add)
                # (M)^{-1/2} for M=[[a,b],[b,c]]: s=sqrt(ac-b^2);
                # t=sqrt(a+c+2s); W=[[c+s,-b],[-b,a+s]]/(s*t)
                det, s_, tr, st, inv = ts(6), ts(7), ts(8), ts(9), ts(10)
                V.tensor_mul(det, a, cc)
                V.tensor_mul(u, bb, bb)
                V.tensor_sub(det, det, u)
                nc.scalar.sqrt(s_, det)
                V.tensor_add(tr, a, cc)
                V.scalar_tensor_tensor(out=tr, in0=s_, scalar=2.0, in1=tr,
                                       op0=Alu.mult, op1=Alu.add)
                nc.scalar.sqrt(tr, tr)
                V.tensor_mul(st, s_, tr)
                V.reciprocal(inv, st)
                w00, w01, w11, q, r = ts(11), ts(12), ts(13), ts(14), ts(15)
                V.tensor_add(u, cc, s_)
                V.tensor_mul(w00, u, inv)
                V.scalar_tensor_tensor(out=w01, in0=bb, scalar=-1.0, in1=inv,
                                       op0=Alu.mult, op1=Alu.mult)
                V.tensor_add(u, a, s_)
                V.tensor_mul(w11, u, inv)
                # G = gamma @ W ; B' = beta - G @ mean  (gamma as [P,2] pairs)
                g00 = gb[:, 0 * 8 + 2 * pr: 0 * 8 + 2 * pr + 2]
                g01 = gb[:, 1 * 8 + 2 * pr: 1 * 8 + 2 * pr + 2]
                g10 = gb[:, 2 * 8 + 2 * pr: 2 * 8 + 2 * pr + 2]
                g11 = gb[:, 3 * 8 + 2 * pr: 3 * 8 + 2 * pr + 2]
                br_ = gb[:, 4 * 8 + 2 * pr: 4 * 8 + 2 * pr + 2]
                bi_ = gb[:, 5 * 8 + 2 * pr: 5 * 8 + 2 * pr + 2]
                cb = smallp.tile([P, 6, 2], f32, tag="cb")
                G00, G01, BR = cb[:, 0, :], cb[:, 1, :], cb[:, 2, :]
                G10, G11, BI = cb[:, 3, :], cb[:, 4, :], cb[:, 5, :]
                V.tensor_mul(q, g00, w00)
                V.tensor_mul(r, g01, w01)
                V.tensor_add(G00, q, r)
                V.tensor_mul(q, g00, w01)
                V.tensor_mul(r, g01, w11)
                V.tensor_add(G01, q, r)
                V.tensor_mul(q, g10, w00)
                V.tensor_mul(r, g11, w01)
                V.tensor_add(G10, q, r)
                V.tensor_mul(q, g10, w01)
                V.tensor_mul(r, g11, w11)
                V.tensor_add(G11, q, r)
                V.tensor_mul(q, G00, MR)
                V.tensor_mul(r, G01, MI)
                V.tensor_add(q, q, r)
                V.tensor_sub(BR, br_, q)
                V.tensor_mul(q, G10, MR)
                V.tensor_mul(r, G11, MI)
                V.tensor_add(q, q, r)
                V.tensor_sub(BI, bi_, q)

                # ScalarE-local copy of the coefficients: the whiten ACT ops
                # read scale/bias from cbS, produced on ACT itself, so they
                # are safe under same-engine program order.
                cbS = smallp.tile([P, 12], f32, tag="cbS")
                nc.scalar.copy(cbS[:], cb[:].rearrange("p a b -> p (a b)"))

                # ---- whiten + affine, per channel of the pair ----
                for i in range(2):
                    c = 2 * pr + i
                    x3 = xt[:, i, :].rearrange("p (j k) -> p j k", k=BLK)
                    xr = x3[:, :, 0:CHUNK]
                    xi = x3[:, :, CHUNK:2 * CHUNK]
                    yt = yp.tile([P, 2, NCHUNK, CHUNK], bf16, tag="y")
                    t1 = tp.tile([P, NCHUNK, CHUNK], bf16, tag="t1")
                    t2 = tp.tile([P, NCHUNK, CHUNK], bf16, tag="t2")
                    nc.scalar.activation(out=t1[:], in_=xr, func=Act.Identity,
                                         scale=cbS[:, 0 + i: 1 + i],
                                         bias=cbS[:, 4 + i: 5 + i])
                    V.scalar_tensor_tensor(out=yt[:, 0], in0=xi,
                                           scalar=cb[:, 1, i: i + 1],
                                           in1=t1[:], op0=Alu.mult,
                                           op1=Alu.add)
                    nc.scalar.activation(out=t2[:], in_=xr, func=Act.Identity,
                                         scale=cbS[:, 6 + i: 7 + i],
                                         bias=cbS[:, 10 + i: 11 + i])
                    V.scalar_tensor_tensor(out=yt[:, 1], in0=xi,
                                           scalar=cb[:, 4, i: i + 1],
                                           in1=t2[:], op0=Alu.mult,
                                           op1=Alu.add)
                    nc.gpsimd.dma_start(
                        out=y_d[c],
                        in_=yt[:].rearrange("p a j k -> p (a j k)"))

    nc.finalize()
    return nc


def _get_nc():
    if "nc" not in _CACHE:
        _CACHE["nc"] = _build_nc()
    return _CACHE["nc"]


def _f32_to_bf16_u16(a):
    """Round-to-nearest-even f32 -> bf16 bit pattern (uint16)."""
    u = np.ascontiguousarray(a, dtype=np.float32).view(np.uint32)
    r = (u + np.uint32(0x7FFF) + ((u >> np.uint32(16)) & np.uint32(1)))
    return (r >> np.uint32(16)).astype(np.uint16)


def _prep_mask():
    m = np.zeros((P, 128), np.float32)
    idx = np.arange(128)
    m[idx, idx] = 1.0
    m[idx[:64], 64 + idx[:64]] = 1.0
    return np.tile(m, (1, 2))


def _prep_core(x_real, x_imag, gamma, beta, k, mask):
    c0 = k * CLOC
    xr = np.ascontiguousarray(
        x_real[:, c0:c0 + CLOC].transpose(1, 0, 2, 3)
    ).reshape(CLOC, P, NCHUNK, CHUNK)
    xi = np.ascontiguousarray(
        x_imag[:, c0:c0 + CLOC].transpose(1, 0, 2, 3)
    ).reshape(CLOC, P, NCHUNK, CHUNK)
    xy = np.empty((CLOC, P, NCHUNK, BLK), np.uint16)
    xy[..., 0:CHUNK] = _f32_to_bf16_u16(xr)
    xy[..., CHUNK:2 * CHUNK] = _f32_to_bf16_u16(xi)
    xy[..., 2 * CHUNK] = 0x3F80      # 1.0 in bf16
    xy[..., 2 * CHUNK + 1] = 0
    # [CLOC, P, NCHUNK, BLK] -> pairs [NPAIR, P, 2, XYW]
    xy = xy.reshape(NPAIR, 2, P, XYW).transpose(0, 2, 1, 3)
    xy = np.ascontiguousarray(xy).reshape(NPAIR, P, 2 * XYW)
    g = gamma[c0:c0 + CLOC]
    b = beta[c0:c0 + CLOC]
    gb = np.concatenate([g[:, 0, 0], g[:, 0, 1], g[:, 1, 0], g[:, 1, 1],
                         b[:, 0], b[:, 1]]).astype(np.float32).reshape(1, 48)
    gb = np.broadcast_to(gb, (P, 48)).copy()
    return {"xy": xy.view(ml_dtypes.bfloat16), "mask": mask, "gb": gb}


def kernel(x_real, x_imag, gamma, beta):
    from concourse.bass_utils import run_bass_kernel_spmd

    x_real = np.asarray(x_real, dtype=np.float32)
    x_imag = np.asarray(x_imag, dtype=np.float32)
    gamma = np.asarray(gamma, dtype=np.float32)
    beta = np.asarray(beta, dtype=np.float32)

    mask = _prep_mask()
    in_maps = [_prep_core(x_real, x_imag, gamma, beta, k, mask)
               for k in range(NCORES)]

    nc = _get_nc()
    res = None
    if _TRACE:
        try:
            res = run_bass_kernel_spmd(nc, in_maps, list(range(NCORES)),
                                       trace=True)
        except Exception as e:  # trace infra unavailable -> plain run
            LAST["trace_error"] = repr(e)
            res = None
    if res is None:
        res = run_bass_kernel_spmd(nc, in_maps, list(range(NCORES)))
    LAST["exec_time_ns"] = res.exec_time_ns
    LAST["mean_exec_time_ns"] = res.mean_exec_time_ns
    LAST["profile_json"] = res.profile_json

    out = np.empty((B, C, H, W, 2), np.float32)
    for k in range(NCORES):
        c0 = k * CLOC
        yb = np.asarray(res.results[k]["y"]).view(np.uint16)
        yf = (yb.astype(np.uint32) << np.uint32(16)).view(np.float32)
        # [CLOC, P, 2, F] -> (B, H, W) per channel/component
        yf = yf.reshape(CLOC, P, 2, F).transpose(0, 2, 1, 3)
        yf = yf.reshape(CLOC, 2, B, H, W).transpose(2, 0, 3, 4, 1)
        out[:, c0:c0 + CLOC] = yf
    return out


# revision 5
# speedup vs baseline: 1.5943x; 1.0659x over previous
"""ComplexBatchNorm2d (Trabelsi-style complex whitening BN) on 8 trn2 NeuronCores.

Sharding: over channels C (8 channels per core); each channel's batch stats are
computed wholly on one core, so no collectives.

bf16 end-to-end: inputs are rounded to bf16 on the host (round-to-nearest-even)
and shipped in a per-pair blocked-interleaved layout [Xr(64)|Xi(64)|1|0] x 64
chunks; outputs leave the device as bf16 planes and are widened to f32 on the
host. This halves HBM traffic (the kernel is memory-bound). Whiten math is
decomposed around measured DVE perf modes: tensor_scalar runs 4x and
tensor_tensor 2x in bf16, while scalar_tensor_tensor and ACT activations are
stuck at 1x. So per channel:
    u[comp] = G[comp,0]*xr + B[comp]   (ACT Identity for 6 channels,
                                        DVE tensor_scalar for 2)
    v[comp] = G[comp,1]*xi             (DVE tensor_scalar, 4x)
    y       = u + v                    (one fused DVE tensor_tensor over both
                                        components, 2x, in-place into v)
Stats per channel pair go through one PSUM gram accumulation; extraction,
partition-fold and the closed-form 2x2 inverse-sqrt solve are batched over
channel QUADS (FD=4 vectors); the solve's elementwise chain runs on GPSIMD
(otherwise idle) with sqrts on ACT and the reciprocal on DVE.

Host side: slices/permutes inputs per core, rounds to bf16, builds the blocked
layout, widens per-core outputs and permutes back to (B, C, H, W, 2) f32.
"""

import numpy as np
import ml_dtypes

# Problem geometry (hardcoded per contract).
B, C, H, W = 32, 64, 128, 128
NCORES = 8
CLOC = C // NCORES          # channels per core = 8
NPAIR = CLOC // 2           # channel pairs per core = 4
NQUAD = CLOC // 4           # channel quads per core = 2
P = 128                     # SBUF partitions
N = B * H * W               # samples per channel = 524288
F = N // P                  # free columns per channel plane = 4096
CHUNK = 64                  # data columns per gram chunk
NCHUNK = F // CHUNK         # 64 chunks per channel
BLK = 2 * CHUNK + 2         # [Xr(64)|Xi(64)|1|pad] = 130 cols per chunk
XYW = NCHUNK * BLK          # 8320 cols per channel (blocked layout)
F2 = 2 * F                  # 8192 output cols per channel (re plane | im plane)
EPS = 1e-5

# channels whose u-ops run on DVE instead of ACT (engine balancing)
DVE_U_CHANNELS = (6, 7)

_CACHE = {}
_TRACE = False   # test.py sets this to capture NTFF profile / HW exec time
LAST = {}        # kernel() stores exec_time_ns etc. here


def _build_nc():
    import concourse.bacc as bacc
    import concourse.mybir as mybir
    from concourse.tile import TileContext

    f32 = mybir.dt.float32
    bf16 = mybir.dt.bfloat16
    Alu = mybir.AluOpType
    Act = mybir.ActivationFunctionType
    Axis = mybir.AxisListType

    nc = bacc.Bacc("TRN2", target_bir_lowering=False)
    xy_d = nc.declare_dram_parameter("xy", [NPAIR, P, 2 * XYW], bf16,
                                     isOutput=False)
    mask_d = nc.declare_dram_parameter("mask", [P, 512], f32, isOutput=False)
    gb_d = nc.declare_dram_parameter("gb", [P, 48], f32, isOutput=False)
    y_d = nc.declare_dram_parameter("y", [CLOC, P, F2], bf16, isOutput=True)

    V = nc.vector
    G = nc.gpsimd
    rN = 1.0 / N
    rN1 = 1.0 / (N - 1)

    with TileContext(nc) as tc:
        with (
            tc.tile_pool(name="singles", bufs=1) as singles,
            tc.tile_pool(name="xyp", bufs=3) as xyp,
            tc.tile_pool(name="up", bufs=2) as up,
            tc.tile_pool(name="vp", bufs=2) as vp,
            tc.tile_pool(name="smallp", bufs=2) as smallp,
            tc.tile_pool(name="gramp", bufs=1, space="PSUM") as gramp,
            tc.tile_pool(name="spsum", bufs=2, space="PSUM") as spsump,
        ):
            mask = singles.tile([P, 4, 128], f32)
            nc.sync.dma_start(out=mask[:].rearrange("p a b -> p (a b)"),
                              in_=mask_d[:])
            gb = singles.tile([P, 48], f32)
            nc.sync.dma_start(out=gb[:], in_=gb_d[:])
            # Full 128-wide ones weights for the partition-fold matmul
            # (fp32 matmuls must keep all PE column groups active).
            ones_mat = singles.tile([P, P], f32)
            V.memset(ones_mat[:], 1.0)

            # All input DMAs issued up front on the sync queue; the tile pool
            # back-pressures pair 3 until pair 0's data is consumed.
            xts = []
            for pr in range(NPAIR):
                xt = xyp.tile([P, 2, XYW], bf16, tag="xy")
                nc.sync.dma_start(out=xt[:].rearrange("p a b -> p (a b)"),
                                  in_=xy_d[pr])
                xts.append(xt)

            for q in range(NQUAD):
                # ---- gram stats: 4 channels into one 4-bank psum ----
                g4 = gramp.tile([P, 4, 512], f32, tag="gram")
                for i in range(4):
                    xt = xts[2 * q + i // 2]
                    ii = i % 2
                    for j in range(NCHUNK):
                        nc.tensor.matmul(
                            g4[:, i, 0:BLK],
                            lhsT=xt[:, ii, j * BLK: j * BLK + 2 * CHUNK],
                            rhs=xt[:, ii, j * BLK: j * BLK + BLK],
                            start=(j == 0),
                            stop=(j == NCHUNK - 1),
                        )

                # ---- batched diag/sum extraction for the quad ----
                stats = smallp.tile([P, 8, 4], f32, tag="stats")
                V.memset(stats[:], 0.0)
                junk = smallp.tile([P, 4, 128], f32, tag="junk")
                V.tensor_mul(junk[:], g4[:, :, 0:128], mask[:])
                V.tensor_reduce(out=stats[:, 0, :], in_=junk[:, :, 0:CHUNK],
                                axis=Axis.X, op=Alu.add)
                V.tensor_reduce(out=stats[0:CHUNK, 1, :],
                                in_=junk[0:CHUNK, :, CHUNK:2 * CHUNK],
                                axis=Axis.X, op=Alu.add)
                V.tensor_reduce(out=stats[CHUNK:P, 2, :],
                                in_=junk[CHUNK:P, :, CHUNK:2 * CHUNK],
                                axis=Axis.X, op=Alu.add)
                V.tensor_copy(stats[0:CHUNK, 3, :],
                              g4[0:CHUNK, :, 2 * CHUNK])
                V.tensor_copy(stats[CHUNK:P, 4, :],
                              g4[CHUNK:P, :, 2 * CHUNK])

                # partition fold: every psum row ends up with the totals
                s_ps = spsump.tile([P, 32], f32, tag="sps")
                nc.tensor.matmul(s_ps[:], lhsT=ones_mat[:],
                                 rhs=stats[:].rearrange("p a b -> p (a b)"),
                                 start=True, stop=True)
                s_sb = smallp.tile([P, 8, 4], f32, tag="ssb")
                V.tensor_copy(s_sb[:].rearrange("p a b -> p (a b)"), s_ps[:])

                # ---- 2x2 assembly on GPSIMD, replicated across partitions,
                #      batched over the channel quad (FD=4 vectors) ----
                SXX, SXY, SYY = s_sb[:, 0, :], s_sb[:, 1, :], s_sb[:, 2, :]
                SR, SI = s_sb[:, 3, :], s_sb[:, 4, :]
                tmp = smallp.tile([P, 16, 4], f32, tag="tmp")

                def ts(i, tmp=tmp):
                    return tmp[:, i, :]

                MR, MI, u = ts(0), ts(1), ts(2)
                a, bb, cc = ts(3), ts(4), ts(5)
                G.tensor_scalar_mul(MR, SR, rN)
                G.tensor_scalar_mul(MI, SI, rN)
                G.tensor_mul(u, SR, MR)
                G.tensor_sub(a, SXX, u)
                G.tensor_scalar(out=a, in0=a, scalar1=rN1, scalar2=EPS,
                                op0=Alu.mult, op1=Alu.add)
                G.tensor_mul(u, SR, MI)
                G.tensor_sub(bb, SXY, u)
                G.tensor_scalar_mul(bb, bb, rN1)
                G.tensor_mul(u, SI, MI)
                G.tensor_sub(cc, SYY, u)
                G.tensor_scalar(out=cc, in0=cc, scalar1=rN1, scalar2=EPS,
                                op0=Alu.mult, op1=Alu.add)
                # (M)^{-1/2} for M=[[a,b],[b,c]]: s=sqrt(ac-b^2);
                # t=sqrt(a+c+2s); W=[[c+s,-b],[-b,a+s]]/(s*t)
                det, s_, tr, st, inv = ts(6), ts(7), ts(8), ts(9), ts(10)
                G.tensor_mul(det, a, cc)
                G.tensor_mul(u, bb, bb)
                G.tensor_sub(det, det, u)
                nc.scalar.sqrt(s_, det)
                G.tensor_add(tr, a, cc)
                G.tensor_add(tr, tr, s_)
                G.tensor_add(tr, tr, s_)
                nc.scalar.sqrt(tr, tr)
                G.tensor_mul(st, s_, tr)
                V.reciprocal(inv, st)
                w00, w01, w11, q_, r_ = ts(11), ts(12), ts(13), ts(14), ts(15)
                G.tensor_add(u, cc, s_)
                G.tensor_mul(w00, u, inv)
                G.tensor_mul(w01, bb, inv)
                G.tensor_scalar_mul(w01, w01, -1.0)
                G.tensor_add(u, a, s_)
                G.tensor_mul(w11, u, inv)
                # G = gamma @ W ; B' = beta - G @ mean  (gamma as [P,4] quads)
                g00 = gb[:, 0 * 8 + 4 * q: 0 * 8 + 4 * q + 4]
                g01 = gb[:, 1 * 8 + 4 * q: 1 * 8 + 4 * q + 4]
                g10 = gb[:, 2 * 8 + 4 * q: 2 * 8 + 4 * q + 4]
                g11 = gb[:, 3 * 8 + 4 * q: 3 * 8 + 4 * q + 4]
                br_ = gb[:, 4 * 8 + 4 * q: 4 * 8 + 4 * q + 4]
                bi_ = gb[:, 5 * 8 + 4 * q: 5 * 8 + 4 * q + 4]
                cb = smallp.tile([P, 6, 4], f32, tag="cb")
                G00, G01, BR = cb[:, 0, :], cb[:, 1, :], cb[:, 2, :]
                G10, G11, BI = cb[:, 3, :], cb[:, 4, :], cb[:, 5, :]
                G.tensor_mul(q_, g00, w00)
                G.tensor_mul(r_, g01, w01)
                G.tensor_add(G00, q_, r_)
                G.tensor_mul(q_, g00, w01)
                G.tensor_mul(r_, g01, w11)
                G.tensor_add(G01, q_, r_)
                G.tensor_mul(q_, g10, w00)
                G.tensor_mul(r_, g11, w01)
                G.tensor_add(G10, q_, r_)
                G.tensor_mul(q_, g10, w01)
                G.tensor_mul(r_, g11, w11)
                G.tensor_add(G11, q_, r_)
                G.tensor_mul(q_, G00, MR)
                G.tensor_mul(r_, G01, MI)
                G.tensor_add(q_, q_, r_)
                G.tensor_sub(BR, br_, q_)
                G.tensor_mul(q_, G10, MR)
                G.tensor_mul(r_, G11, MI)
                G.tensor_add(q_, q_, r_)
                G.tensor_sub(BI, bi_, q_)

                # Same-engine staging copies of the coefficients: ACT reads
                # scale/bias from cbS (ACT-written), DVE reads scalars from
                # cbV (DVE-written) — safe under same-engine program order.
                cbS = smallp.tile([P, 24], f32, tag="cbS")
                nc.scalar.copy(cbS[:], cb[:].rearrange("p a b -> p (a b)"))
                cbV = smallp.tile([P, 6, 4], f32, tag="cbV")
                V.tensor_copy(cbV[:].rearrange("p a b -> p (a b)"),
                              cb[:].rearrange("p a b -> p (a b)"))

                # ---- whiten + affine, per channel of the quad ----
                for i in range(4):
                    c = 4 * q + i
                    xt = xts[2 * q + i // 2]
                    x3 = xt[:, i % 2, :].rearrange("p (j k) -> p j k", k=BLK)
                    xr = x3[:, :, 0:CHUNK]
                    xi = x3[:, :, CHUNK:2 * CHUNK]
                    ut = up.tile([P, 2, NCHUNK, CHUNK], bf16, tag="u")
                    vt = vp.tile([P, 2, NCHUNK, CHUNK], bf16, tag="v")
                    # u[comp] = G[comp,0]*xr + B[comp]
                    if c in DVE_U_CHANNELS:
                        V.tensor_scalar(out=ut[:, 0], in0=xr,
                                        scalar1=cbV[:, 0, i: i + 1],
                                        scalar2=cbV[:, 2, i: i + 1],
                                        op0=Alu.mult, op1=Alu.add)
                        V.tensor_scalar(out=ut[:, 1], in0=xr,
                                        scalar1=cbV[:, 3, i: i + 1],
                                        scalar2=cbV[:, 5, i: i + 1],
                                        op0=Alu.mult, op1=Alu.add)
                    else:
                        nc.scalar.activation(out=ut[:, 0], in_=xr,
                                             func=Act.Identity,
                                             scale=cbS[:, 0 * 4 + i: 0 * 4 + i + 1],
                                             bias=cbS[:, 2 * 4 + i: 2 * 4 + i + 1])
                        nc.scalar.activation(out=ut[:, 1], in_=xr,
                                             func=Act.Identity,
                                             scale=cbS[:, 3 * 4 + i: 3 * 4 + i + 1],
                                             bias=cbS[:, 5 * 4 + i: 5 * 4 + i + 1])
                    # v[comp] = G[comp,1]*xi   (4x tensor_scalar)
                    V.tensor_scalar(out=vt[:, 0], in0=xi,
                                    scalar1=cbV[:, 1, i: i + 1], scalar2=None,
                                    op0=Alu.mult)
                    V.tensor_scalar(out=vt[:, 1], in0=xi,
                                    scalar1=cbV[:, 4, i: i + 1], scalar2=None,
                                    op0=Alu.mult)
                    # y = u + v, both components fused (2x tensor_tensor)
                    V.tensor_tensor(out=vt[:], in0=ut[:], in1=vt[:],
                                    op=Alu.add)
                    nc.sync.dma_start(
                        out=y_d[c],
                        in_=vt[:].rearrange("p a j k -> p (a j k)"))

    nc.finalize()
    return nc


def _get_nc():
    if "nc" not in _CACHE:
        _CACHE["nc"] = _build_nc()
    return _CACHE["nc"]


def _f32_to_bf16_u16(a):
    """Round-to-nearest-even f32 -> bf16 bit pattern (uint16)."""
    u = np.ascontiguousarray(a, dtype=np.float32).view(np.uint32)
    r = (u + np.uint32(0x7FFF) + ((u >> np.uint32(16)) & np.uint32(1)))
    return (r >> np.uint32(16)).astype(np.uint16)


def _prep_mask():
    m = np.zeros((P, 128), np.float32)
    idx = np.arange(128)
    m[idx, idx] = 1.0
    m[idx[:64], 64 + idx[:64]] = 1.0
    return np.tile(m, (1, 4))


def _prep_core(x_real, x_imag, gamma, beta, k, mask):
    c0 = k * CLOC
    xr = np.ascontiguousarray(
        x_real[:, c0:c0 + CLOC].transpose(1, 0, 2, 3)
    ).reshape(CLOC, P, NCHUNK, CHUNK)
    xi = np.ascontiguousarray(
        x_imag[:, c0:c0 + CLOC].transpose(1, 0, 2, 3)
    ).reshape(CLOC, P, NCHUNK, CHUNK)
    xy = np.empty((CLOC, P, NCHUNK, BLK), np.uint16)
    xy[..., 0:CHUNK] = _f32_to_bf16_u16(xr)
    xy[..., CHUNK:2 * CHUNK] = _f32_to_bf16_u16(xi)
    xy[..., 2 * CHUNK] = 0x3F80      # 1.0 in bf16
    xy[..., 2 * CHUNK + 1] = 0
    # [CLOC, P, NCHUNK, BLK] -> pairs [NPAIR, P, 2, XYW]
    xy = xy.reshape(NPAIR, 2, P, XYW).transpose(0, 2, 1, 3)
    xy = np.ascontiguousarray(xy).reshape(NPAIR, P, 2 * XYW)
    g = gamma[c0:c0 + CLOC]
    b = beta[c0:c0 + CLOC]
    gb = np.concatenate([g[:, 0, 0], g[:, 0, 1], g[:, 1, 0], g[:, 1, 1],
                         b[:, 0], b[:, 1]]).astype(np.float32).reshape(1, 48)
    gb = np.broadcast_to(gb, (P, 48)).copy()
    return {"xy": xy.view(ml_dtypes.bfloat16), "mask": mask, "gb": gb}


def kernel(x_real, x_imag, gamma, beta):
    from concourse.bass_utils import run_bass_kernel_spmd

    x_real = np.asarray(x_real, dtype=np.float32)
    x_imag = np.asarray(x_imag, dtype=np.float32)
    gamma = np.asarray(gamma, dtype=np.float32)
    beta = np.asarray(beta, dtype=np.float32)

    mask = _prep_mask()
    in_maps = [_prep_core(x_real, x_imag, gamma, beta, k, mask)
               for k in range(NCORES)]

    nc = _get_nc()
    res = None
    if _TRACE:
        try:
            res = run_bass_kernel_spmd(nc, in_maps, list(range(NCORES)),
                                       trace=True)
        except Exception as e:  # trace infra unavailable -> plain run
            LAST["trace_error"] = repr(e)
            res = None
    if res is None:
        res = run_bass_kernel_spmd(nc, in_maps, list(range(NCORES)))
    LAST["exec_time_ns"] = res.exec_time_ns
    LAST["mean_exec_time_ns"] = res.mean_exec_time_ns
    LAST["profile_json"] = res.profile_json

    out = np.empty((B, C, H, W, 2), np.float32)
    for k in range(NCORES):
        c0 = k * CLOC
        yb = np.asarray(res.results[k]["y"]).view(np.uint16)
        yf = (yb.astype(np.uint32) << np.uint32(16)).view(np.float32)
        # [CLOC, P, 2, F] -> (B, H, W) per channel/component
        yf = yf.reshape(CLOC, P, 2, F).transpose(0, 2, 1, 3)
        yf = yf.reshape(CLOC, 2, B, H, W).transpose(2, 0, 3, 4, 1)
        out[:, c0:c0 + CLOC] = yf
    return out


# revision 7
# speedup vs baseline: 1.8769x; 1.1773x over previous
"""ComplexBatchNorm2d (Trabelsi-style complex whitening BN) on 8 trn2 NeuronCores.

Sharding: over channels C (8 channels per core); each channel's batch stats are
computed wholly on one core, so no collectives.

bf16 end-to-end: inputs are rounded to bf16 on the host (round-to-nearest-even)
and shipped in a per-pair blocked-interleaved layout [Xr(64)|Xi(64)|1|0] x 64
chunks; outputs leave the device as bf16 planes and are widened to f32 on the
host. This halves HBM traffic (the kernel is memory-bound). Whiten math is
decomposed around measured DVE perf modes: tensor_scalar runs 4x and
tensor_tensor 2x in bf16, while scalar_tensor_tensor and ACT activations are
stuck at 1x. So per channel:
    u[comp] = G[comp,0]*xr + B[comp]   (ACT Identity for 6 channels,
                                        DVE tensor_scalar for 2)
    v[comp] = G[comp,1]*xi             (DVE tensor_scalar, 4x)
    y       = u + v                    (one fused DVE tensor_tensor over both
                                        components, 2x, in-place into v)
Stats per channel pair go through one PSUM gram accumulation; extraction,
partition-fold and the closed-form 2x2 inverse-sqrt solve are batched over
channel QUADS (FD=4 vectors); the solve's elementwise chain runs on GPSIMD
(otherwise idle) with sqrts on ACT and the reciprocal on DVE.

Host side: slices/permutes inputs per core, rounds to bf16, builds the blocked
layout, widens per-core outputs and permutes back to (B, C, H, W, 2) f32.
"""

import numpy as np
import ml_dtypes

# Problem geometry (hardcoded per contract).
B, C, H, W = 32, 64, 128, 128
NCORES = 8
CLOC = C // NCORES          # channels per core = 8
NPAIR = CLOC // 2           # channel pairs per core = 4
NQUAD = CLOC // 4           # channel quads per core = 2
P = 128                     # SBUF partitions
N = B * H * W               # samples per channel = 524288
F = N // P                  # free columns per channel plane = 4096
CHUNK = 64                  # data columns per gram chunk
NCHUNK = F // CHUNK         # 64 chunks per channel
BLK = 2 * CHUNK + 2         # [Xr(64)|Xi(64)|1|pad] = 130 cols per chunk
XYW = NCHUNK * BLK          # 8320 cols per channel (blocked layout)
F2 = 2 * F                  # 8192 output cols per channel (re plane | im plane)
EPS = 1e-5

# channels whose u-ops run on DVE instead of ACT (engine balancing)
DVE_U_CHANNELS = (6, 7)

_CACHE = {}
_TRACE = False   # test.py sets this to capture NTFF profile / HW exec time
LAST = {}        # kernel() stores exec_time_ns etc. here


def _build_nc():
    import concourse.bacc as bacc
    import concourse.mybir as mybir
    from concourse.tile import TileContext

    f32 = mybir.dt.float32
    bf16 = mybir.dt.bfloat16
    Alu = mybir.AluOpType
    Act = mybir.ActivationFunctionType
    Axis = mybir.AxisListType

    nc = bacc.Bacc("TRN2", target_bir_lowering=False)
    xy_d = nc.declare_dram_parameter("xy", [NPAIR, P, 2 * XYW], bf16,
                                     isOutput=False)
    mask_d = nc.declare_dram_parameter("mask", [P, 512], f32, isOutput=False)
    gb_d = nc.declare_dram_parameter("gb", [P, 48], f32, isOutput=False)
    y_d = nc.declare_dram_parameter("y", [CLOC, P, F2], bf16, isOutput=True)

    V = nc.vector
    G = nc.gpsimd
    rN = 1.0 / N
    rN1 = 1.0 / (N - 1)

    with TileContext(nc) as tc:
        with (
            tc.tile_pool(name="singles", bufs=1) as singles,
            tc.tile_pool(name="xyp", bufs=4) as xyp,
            tc.tile_pool(name="up", bufs=2) as up,
            tc.tile_pool(name="vp", bufs=2) as vp,
            tc.tile_pool(name="smallp", bufs=2) as smallp,
            tc.tile_pool(name="gramp", bufs=1, space="PSUM") as gramp,
            tc.tile_pool(name="spsum", bufs=2, space="PSUM") as spsump,
        ):
            mask = singles.tile([P, 4, 128], f32)
            nc.sync.dma_start(out=mask[:].rearrange("p a b -> p (a b)"),
                              in_=mask_d[:])
            gb = singles.tile([P, 48], f32)
            nc.sync.dma_start(out=gb[:], in_=gb_d[:])
            # Full 128-wide ones weights for the partition-fold matmul
            # (fp32 matmuls must keep all PE column groups active).
            ones_mat = singles.tile([P, P], f32)
            V.memset(ones_mat[:], 1.0)

            # All input DMAs issued up front on the sync queue; the tile pool
            # back-pressures pair 3 until pair 0's data is consumed.
            xts = []
            for pr in range(NPAIR):
                xt = xyp.tile([P, 2, XYW], bf16, tag="xy")
                nc.sync.dma_start(out=xt[:].rearrange("p a b -> p (a b)"),
                                  in_=xy_d[pr])
                xts.append(xt)

            for q in range(NQUAD):
                # ---- gram stats: 4 channels into one 4-bank psum ----
                g4 = gramp.tile([P, 4, 512], f32, tag="gram")
                for i in range(4):
                    xt = xts[2 * q + i // 2]
                    ii = i % 2
                    for j in range(NCHUNK):
                        nc.tensor.matmul(
                            g4[:, i, 0:BLK],
                            lhsT=xt[:, ii, j * BLK: j * BLK + 2 * CHUNK],
                            rhs=xt[:, ii, j * BLK: j * BLK + BLK],
                            start=(j == 0),
                            stop=(j == NCHUNK - 1),
                        )

                # ---- batched diag/sum extraction for the quad ----
                stats = smallp.tile([P, 8, 4], f32, tag="stats")
                V.memset(stats[:], 0.0)
                junk = smallp.tile([P, 4, 128], f32, tag="junk")
                V.tensor_mul(junk[:], g4[:, :, 0:128], mask[:])
                V.tensor_reduce(out=stats[:, 0, :], in_=junk[:, :, 0:CHUNK],
                                axis=Axis.X, op=Alu.add)
                V.tensor_reduce(out=stats[0:CHUNK, 1, :],
                                in_=junk[0:CHUNK, :, CHUNK:2 * CHUNK],
                                axis=Axis.X, op=Alu.add)
                V.tensor_reduce(out=stats[CHUNK:P, 2, :],
                                in_=junk[CHUNK:P, :, CHUNK:2 * CHUNK],
                                axis=Axis.X, op=Alu.add)
                V.tensor_copy(stats[0:CHUNK, 3, :],
                              g4[0:CHUNK, :, 2 * CHUNK])
                V.tensor_copy(stats[CHUNK:P, 4, :],
                              g4[CHUNK:P, :, 2 * CHUNK])

                # partition fold: every psum row ends up with the totals
                s_ps = spsump.tile([P, 32], f32, tag="sps")
                nc.tensor.matmul(s_ps[:], lhsT=ones_mat[:],
                                 rhs=stats[:].rearrange("p a b -> p (a b)"),
                                 start=True, stop=True)
                s_sb = smallp.tile([P, 8, 4], f32, tag="ssb")
                V.tensor_copy(s_sb[:].rearrange("p a b -> p (a b)"), s_ps[:])

                # ---- 2x2 assembly on GPSIMD, replicated across partitions,
                #      batched over the channel quad (FD=4 vectors) ----
                SXX, SXY, SYY = s_sb[:, 0, :], s_sb[:, 1, :], s_sb[:, 2, :]
                SR, SI = s_sb[:, 3, :], s_sb[:, 4, :]
                tmp = smallp.tile([P, 16, 4], f32, tag="tmp")

                def ts(i, tmp=tmp):
                    return tmp[:, i, :]

                # quad 0's solve is on the critical path -> DVE (lower op
                # latency); quad 1's overlaps whiten of quad 0 -> GPSIMD.
                E = V if q == 0 else G

                MR, MI, u = ts(0), ts(1), ts(2)
                a, bb, cc = ts(3), ts(4), ts(5)
                E.tensor_scalar_mul(MR, SR, rN)
                E.tensor_scalar_mul(MI, SI, rN)
                E.tensor_mul(u, SR, MR)
                E.tensor_sub(a, SXX, u)
                E.tensor_scalar(out=a, in0=a, scalar1=rN1, scalar2=EPS,
                                op0=Alu.mult, op1=Alu.add)
                E.tensor_mul(u, SR, MI)
                E.tensor_sub(bb, SXY, u)
                E.tensor_scalar_mul(bb, bb, rN1)
                E.tensor_mul(u, SI, MI)
                E.tensor_sub(cc, SYY, u)
                E.tensor_scalar(out=cc, in0=cc, scalar1=rN1, scalar2=EPS,
                                op0=Alu.mult, op1=Alu.add)
                # (M)^{-1/2} for M=[[a,b],[b,c]]: s=sqrt(ac-b^2);
                # t=sqrt(a+c+2s); W=[[c+s,-b],[-b,a+s]]/(s*t)
                det, s_, tr, st, inv = ts(6), ts(7), ts(8), ts(9), ts(10)
                E.tensor_mul(det, a, cc)
                E.tensor_mul(u, bb, bb)
                E.tensor_sub(det, det, u)
                nc.scalar.sqrt(s_, det)
                E.tensor_add(tr, a, cc)
                E.tensor_add(tr, tr, s_)
                E.tensor_add(tr, tr, s_)
                nc.scalar.sqrt(tr, tr)
                E.tensor_mul(st, s_, tr)
                V.reciprocal(inv, st)
                w00, w01, w11, q_, r_ = ts(11), ts(12), ts(13), ts(14), ts(15)
                E.tensor_add(u, cc, s_)
                E.tensor_mul(w00, u, inv)
                E.tensor_mul(w01, bb, inv)
                E.tensor_scalar_mul(w01, w01, -1.0)
                E.tensor_add(u, a, s_)
                E.tensor_mul(w11, u, inv)
                # G = gamma @ W ; B' = beta - G @ mean  (gamma as [P,4] quads)
                g00 = gb[:, 0 * 8 + 4 * q: 0 * 8 + 4 * q + 4]
                g01 = gb[:, 1 * 8 + 4 * q: 1 * 8 + 4 * q + 4]
                g10 = gb[:, 2 * 8 + 4 * q: 2 * 8 + 4 * q + 4]
                g11 = gb[:, 3 * 8 + 4 * q: 3 * 8 + 4 * q + 4]
                br_ = gb[:, 4 * 8 + 4 * q: 4 * 8 + 4 * q + 4]
                bi_ = gb[:, 5 * 8 + 4 * q: 5 * 8 + 4 * q + 4]
                cb = smallp.tile([P, 6, 4], f32, tag="cb")
                G00, G01, BR = cb[:, 0, :], cb[:, 1, :], cb[:, 2, :]
                G10, G11, BI = cb[:, 3, :], cb[:, 4, :], cb[:, 5, :]
                E.tensor_mul(q_, g00, w00)
                E.tensor_mul(r_, g01, w01)
                E.tensor_add(G00, q_, r_)
                E.tensor_mul(q_, g00, w01)
                E.tensor_mul(r_, g01, w11)
                E.tensor_add(G01, q_, r_)
                E.tensor_mul(q_, g10, w00)
                E.tensor_mul(r_, g11, w01)
                E.tensor_add(G10, q_, r_)
                E.tensor_mul(q_, g10, w01)
                E.tensor_mul(r_, g11, w11)
                E.tensor_add(G11, q_, r_)
                E.tensor_mul(q_, G00, MR)
                E.tensor_mul(r_, G01, MI)
                E.tensor_add(q_, q_, r_)
                E.tensor_sub(BR, br_, q_)
                E.tensor_mul(q_, G10, MR)
                E.tensor_mul(r_, G11, MI)
                E.tensor_add(q_, q_, r_)
                E.tensor_sub(BI, bi_, q_)

                # Same-engine staging copies of the coefficients: ACT reads
                # scale/bias from cbS (ACT-written), DVE reads scalars from
                # cbV (DVE-written) — safe under same-engine program order.
                cbS = smallp.tile([P, 24], f32, tag="cbS")
                nc.scalar.copy(cbS[:], cb[:].rearrange("p a b -> p (a b)"))
                cbV = smallp.tile([P, 6, 4], f32, tag="cbV")
                V.tensor_copy(cbV[:].rearrange("p a b -> p (a b)"),
                              cb[:].rearrange("p a b -> p (a b)"))

                # ---- whiten + affine, per channel of the quad ----
                for i in range(4):
                    c = 4 * q + i
                    xt = xts[2 * q + i // 2]
                    x3 = xt[:, i % 2, :].rearrange("p (j k) -> p j k", k=BLK)
                    xr = x3[:, :, 0:CHUNK]
                    xi = x3[:, :, CHUNK:2 * CHUNK]
                    ut = up.tile([P, 2, NCHUNK, CHUNK], bf16, tag="u")
                    vt = vp.tile([P, 2, NCHUNK, CHUNK], bf16, tag="v")
                    # u[comp] = G[comp,0]*xr + B[comp]
                    if c in DVE_U_CHANNELS:
                        V.tensor_scalar(out=ut[:, 0], in0=xr,
                                        scalar1=cbV[:, 0, i: i + 1],
                                        scalar2=cbV[:, 2, i: i + 1],
                                        op0=Alu.mult, op1=Alu.add)
                        V.tensor_scalar(out=ut[:, 1], in0=xr,
                                        scalar1=cbV[:, 3, i: i + 1],
                                        scalar2=cbV[:, 5, i: i + 1],
                                        op0=Alu.mult, op1=Alu.add)
                    else:
                        nc.scalar.activation(out=ut[:, 0], in_=xr,
                                             func=Act.Identity,
                                             scale=cbS[:, 0 * 4 + i: 0 * 4 + i + 1],
                                             bias=cbS[:, 2 * 4 + i: 2 * 4 + i + 1])
                        nc.scalar.activation(out=ut[:, 1], in_=xr,
                                             func=Act.Identity,
                                             scale=cbS[:, 3 * 4 + i: 3 * 4 + i + 1],
                                             bias=cbS[:, 5 * 4 + i: 5 * 4 + i + 1])
                    # v[comp] = G[comp,1]*xi   (4x tensor_scalar)
                    V.tensor_scalar(out=vt[:, 0], in0=xi,
                                    scalar1=cbV[:, 1, i: i + 1], scalar2=None,
                                    op0=Alu.mult)
                    V.tensor_scalar(out=vt[:, 1], in0=xi,
                                    scalar1=cbV[:, 4, i: i + 1], scalar2=None,
                                    op0=Alu.mult)
                    # y = u + v, both components fused (2x tensor_tensor)
                    V.tensor_tensor(out=vt[:], in0=ut[:], in1=vt[:],
                                    op=Alu.add)
                    nc.sync.dma_start(
                        out=y_d[c],
                        in_=vt[:].rearrange("p a j k -> p (a j k)"))

    nc.finalize()
    return nc


def _get_nc():
    if "nc" not in _CACHE:
        _CACHE["nc"] = _build_nc()
    return _CACHE["nc"]


def _f32_to_bf16_u16(a):
    """Round-to-nearest-even f32 -> bf16 bit pattern (uint16)."""
    u = np.ascontiguousarray(a, dtype=np.float32).view(np.uint32)
    r = (u + np.uint32(0x7FFF) + ((u >> np.uint32(16)) & np.uint32(1)))
    return (r >> np.uint32(16)).astype(np.uint16)


def _prep_mask():
    m = np.zeros((P, 128), np.float32)
    idx = np.arange(128)
    m[idx, idx] = 1.0
    m[idx[:64], 64 + idx[:64]] = 1.0
    return np.tile(m, (1, 4))


def _prep_core(x_real, x_imag, gamma, beta, k, mask):
    c0 = k * CLOC
    xr = np.ascontiguousarray(
        x_real[:, c0:c0 + CLOC].transpose(1, 0, 2, 3)
    ).reshape(CLOC, P, NCHUNK, CHUNK)
    xi = np.ascontiguousarray(
        x_imag[:, c0:c0 + CLOC].transpose(1, 0, 2, 3)
    ).reshape(CLOC, P, NCHUNK, CHUNK)
    xy = np.empty((CLOC, P, NCHUNK, BLK), np.uint16)
    xy[..., 0:CHUNK] = _f32_to_bf16_u16(xr)
    xy[..., CHUNK:2 * CHUNK] = _f32_to_bf16_u16(xi)
    xy[..., 2 * CHUNK] = 0x3F80      # 1.0 in bf16
    xy[..., 2 * CHUNK + 1] = 0
    # [CLOC, P, NCHUNK, BLK] -> pairs [NPAIR, P, 2, XYW]
    xy = xy.reshape(NPAIR, 2, P, XYW).transpose(0, 2, 1, 3)
    xy = np.ascontiguousarray(xy).reshape(NPAIR, P, 2 * XYW)
    g = gamma[c0:c0 + CLOC]
    b = beta[c0:c0 + CLOC]
    gb = np.concatenate([g[:, 0, 0], g[:, 0, 1], g[:, 1, 0], g[:, 1, 1],
                         b[:, 0], b[:, 1]]).astype(np.float32).reshape(1, 48)
    gb = np.broadcast_to(gb, (P, 48)).copy()
    return {"xy": xy.view(ml_dtypes.bfloat16), "mask": mask, "gb": gb}


def kernel(x_real, x_imag, gamma, beta):
    from concourse.bass_utils import run_bass_kernel_spmd

    x_real = np.asarray(x_real, dtype=np.float32)
    x_imag = np.asarray(x_imag, dtype=np.float32)
    gamma = np.asarray(gamma, dtype=np.float32)
    beta = np.asarray(beta, dtype=np.float32)

    mask = _prep_mask()
    in_maps = [_prep_core(x_real, x_imag, gamma, beta, k, mask)
               for k in range(NCORES)]

    nc = _get_nc()
    res = None
    if _TRACE:
        try:
            res = run_bass_kernel_spmd(nc, in_maps, list(range(NCORES)),
                                       trace=True)
        except Exception as e:  # trace infra unavailable -> plain run
            LAST["trace_error"] = repr(e)
            res = None
    if res is None:
        res = run_bass_kernel_spmd(nc, in_maps, list(range(NCORES)))
    LAST["exec_time_ns"] = res.exec_time_ns
    LAST["mean_exec_time_ns"] = res.mean_exec_time_ns
    LAST["profile_json"] = res.profile_json

    out = np.empty((B, C, H, W, 2), np.float32)
    for k in range(NCORES):
        c0 = k * CLOC
        yb = np.asarray(res.results[k]["y"]).view(np.uint16)
        yf = (yb.astype(np.uint32) << np.uint32(16)).view(np.float32)
        # [CLOC, P, 2, F] -> (B, H, W) per channel/component
        yf = yf.reshape(CLOC, P, 2, F).transpose(0, 2, 1, 3)
        yf = yf.reshape(CLOC, 2, B, H, W).transpose(2, 0, 3, 4, 1)
        out[:, c0:c0 + CLOC] = yf
    return out
